# revision 27
# baseline (speedup 1.0000x reference)
"""Trainium2 Bass kernel for max-pooled KNN cache retrieval.

Computes, for each batch element b:
    att[b, n]   = max_{i,j} dot(query[b, i, :], keys[b, n, j, :])   (fp32)
    topk_idx    = indices of the 8 largest att[b, :] (descending)

Sharding: data-parallel over the batch dim (bsz=32 over 8 cores, 4 each).
The `values` input is dead code in the reference module and is never read.

Per-core pipeline (all under the Tile framework):
  1. DMA K rows naturally (nj on partitions, h in free dim, 1KB contiguous).
  2. PE-transpose 128x128 blocks so the contraction dim h lands on partitions.
  3. fp32r matmul  att_tile(64 i, 512 nj) = qT^T @ KT  (PSUM accumulate over h).
  4. DVE max-reduce over j per cache entry -> acc(64 i, 64 n) per batch.
  5. PE-transpose acc, DVE max-reduce over i -> att[b, n].
  6. vector.max / vector.max_index -> exact top-8 (desc, first-index ties).
"""

import os

import numpy as np

# Problem constants (hardcoded per harness contract).
L = 64          # num_steps (query rows i, key rows j per cache entry)
H = 256         # nhid (contraction dim)
N = 64          # cache entries
TOPK = 8
BSZ = 32
NCORES = 8
BPC = BSZ // NCORES   # batches per core = 4
NB = 8                # cache entries per chunk (= 512 K rows = one matmul tile)
NCHUNKS = N // NB     # 8 chunks per batch

_RUNNERS = {}


def _use_fp32r() -> bool:
    return os.environ.get("BASS_KNN_FP32R", "1") == "1"


def _use_f32r_loads() -> bool:
    return os.environ.get("BASS_KNN_F32R_LOADS", "1") == "1"


def _build_nc(use_fp32r: bool, f32r_loads: bool = False):
    import concourse.bacc as bacc
    import concourse.mybir as mybir
    from concourse.masks import make_identity
    from concourse.tile import TileContext

    f32 = mybir.dt.float32
    mm_dt = mybir.dt.float32r if use_fp32r else f32
    kd_dt = mm_dt if f32r_loads else f32

    nc = bacc.Bacc("TRN2", target_bir_lowering=False, debug=False)
    q_d = nc.declare_dram_parameter("q", [L, BPC, H], f32, isOutput=False)
    # per-core keys pre-permuted to [b, n, j, h] so (b, n) merges into a
    # single 128-partition outer DMA dim (full 16-engine fan-out).
    k_d = nc.declare_dram_parameter("k", [BPC, N, L, H], kd_dt, isOutput=False)
    att_d = nc.declare_dram_parameter("att", [BPC, N], f32, isOutput=True)
    idx_d = nc.declare_dram_parameter(
        "idx", [BPC, TOPK], mybir.dt.uint32, isOutput=True
    )

    AX = mybir.AxisListType
    MAX = mybir.AluOpType.max



    with TileContext(nc) as tc:
        with (
            tc.tile_pool(name="cpool", bufs=1) as cpool,
            tc.tile_pool(name="qpool", bufs=1) as qpool,
            tc.tile_pool(name="kpool", bufs=2) as kpool,
            tc.tile_pool(name="ktpool", bufs=1) as ktpool,
            tc.tile_pool(name="opool", bufs=1) as opool,
            tc.tile_pool(name="ps_kt", bufs=2, space="PSUM") as ps_kt,
            tc.tile_pool(name="ps_att", bufs=2, space="PSUM") as ps_att,
            tc.tile_pool(name="ps_sm", bufs=1, space="PSUM") as ps_sm,
        ):
            ident = cpool.tile([128, 128], f32)
            make_identity(nc, ident[:])
            if f32r_loads:
                identr_t = cpool.tile([128, 128], kd_dt, name="identr")
                nc.scalar.copy(identr_t[:], ident[:])
                identr = identr_t[:]
            else:
                identr = ident[:]

            # --- q prep: one DMA for all batches, then per-batch transposes
            qnat = qpool.tile([L, BPC * H], f32, name="qnat", tag="qnat")
            nc.sync.dma_start(out=qnat[:], in_=q_d[:, :, :])
            qTs = []
            for b in range(BPC):
                hs = []
                for hc in range(2):
                    pst = ps_sm.tile([128, L], f32, name="qt_ps", tag="qt_ps")
                    nc.tensor.transpose(
                        pst[:],
                        qnat[:, b * H + hc * 128 : b * H + (hc + 1) * 128],
                        ident[:L, :L],
                    )
                    qt = qpool.tile(
                        [128, L], mm_dt, name=f"qt{b}_{hc}", tag=f"qt{b}_{hc}"
                    )
                    nc.scalar.copy(qt[:], pst[:])
                    hs.append(qt)
                qTs.append(hs)

            accs = [
                opool.tile([L, N], f32, name=f"acc{b}", tag=f"acc{b}")
                for b in range(BPC)
            ]

            # --- main loop: batches processed in pairs. K tile for a pair
            # and j-half: partition = 64*bb + n (bb = batch within pair),
            # free = j_local*256 + h -> each partition holds one entry's
            # contiguous 32KB j-half, so DMA descriptors are 32KB (vs 1KB
            # for an nj-partition layout). Two 2MB DMAs per tile, one per
            # batch band, issued on the two HWDGE rings.
            # PE transposes produce (h, nmix) columns; the PSUM->SBUF copy
            # re-strides them into KT tiles laid out n-major (col = nmix*32
            # + j_local) so matmul rhs slices are contiguous and the j-max
            # reduce keeps its one-op-per-chunk shape.
            JH = L // 2  # j rows per half-tile (32)
            for pair in range(2):
                kts = {}
                for jh in range(2):
                    ksb = kpool.tile([128, JH * H], kd_dt, name="ksb", tag="ksb")
                    # Two 2MB DMAs across all 128 partitions (16KB/partition
                    # descriptors, all 16 SDMA engines), one per HWDGE ring,
                    # split by j range so transposes start after the first.
                    for half, eng in ((0, nc.sync), (1, nc.scalar)):
                        j0 = jh * JH + half * (JH // 2)
                        eng.dma_start(
                            out=ksb[:, half * (JH // 2) * H : (half + 1) * (JH // 2) * H],
                            in_=k_d[
                                pair * 2 : pair * 2 + 2, :, j0 : j0 + JH // 2, :
                            ],
                        )
                    for hc in range(2):
                        kt = ktpool.tile(
                            [128, 128 * JH],
                            mm_dt,
                            name=f"kt{jh}_{hc}",
                            tag=f"kt{jh}_{hc}",
                            bufs=1,
                        )
                        ktv = kt.rearrange("p (n j) -> p n j", n=128)
                        for g in range(JH // 4):
                            ktp = ps_kt.tile(
                                [128, 512], kd_dt, name=f"ktp{hc}", tag=f"ktp{hc}"
                            )
                            for t in range(4):
                                jl = g * 4 + t
                                nc.tensor.transpose(
                                    ktp[:, t * 128 : (t + 1) * 128],
                                    ksb[
                                        :,
                                        jl * H
                                        + hc * 128 : jl * H
                                        + hc * 128
                                        + 128,
                                    ],
                                    identr,
                                )
                            if hc == 0:
                                nc.scalar.copy(
                                    out=ktv[:, :, g * 4 : (g + 1) * 4],
                                    in_=ktp[:].rearrange("p (j n) -> p n j", j=4),
                                )
                            else:
                                nc.vector.tensor_copy(
                                    out=ktv[:, :, g * 4 : (g + 1) * 4],
                                    in_=ktp[:].rearrange("p (j n) -> p n j", j=4),
                                )
                        kts[(jh, hc)] = kt
                # matmuls: per batch, 8 chunks of 8 entries; attp columns =
                # (jh, n_local, j_local); one 4D reduce maxes over (jh, j).
                for bb in range(2):
                    b = pair * 2 + bb
                    for cx in range(8):
                        attp = ps_att.tile([L, 512], f32, name="attp", tag="attp")
                        for jh in range(2):
                            for hc in range(2):
                                nc.tensor.matmul(
                                    attp[:, jh * 256 : (jh + 1) * 256],
                                    qTs[b][hc][:],
                                    kts[(jh, hc)][
                                        :,
                                        bb * 64 * JH
                                        + cx * 256 : bb * 64 * JH
                                        + (cx + 1) * 256,
                                    ],
                                    start=(hc == 0),
                                    stop=(hc == 1),
                                )
                        nc.vector.tensor_reduce(
                            out=accs[b][:, cx * 8 : (cx + 1) * 8],
                            in_=attp[:].rearrange(
                                "p (jh n j) -> p n jh j", jh=2, n=8
                            ),
                            axis=AX.XY,
                            op=MAX,
                        )

            # --- epilogue: max over i, then top-8 per batch
            col = opool.tile([N, BPC], f32)
            for b in range(BPC):
                accT = ps_sm.tile([N, L], f32, name="accT", tag="accT")
                nc.tensor.transpose(accT[:], accs[b][:], ident[:L, :L])
                nc.vector.tensor_reduce(
                    out=col[:, b : b + 1], in_=accT[:], axis=AX.X, op=MAX
                )
            colT = ps_sm.tile([BPC, N], f32, name="colT", tag="accT")
            nc.tensor.transpose(colT[:], col[:], ident[:N, :N])
            attb = opool.tile([BPC, N], f32)
            nc.scalar.copy(attb[:], colT[:])
            mx = opool.tile([BPC, TOPK], f32)
            ix = opool.tile([BPC, TOPK], mybir.dt.uint32)
            nc.vector.max(out=mx[:], in_=attb[:])
            nc.vector.max_index(out=ix[:], in_max=mx[:], in_values=attb[:])
            nc.gpsimd.dma_start(out=att_d[:, :], in_=attb[:])
            nc.gpsimd.dma_start(out=idx_d[:, :], in_=ix[:])

    nc.finalize()
    return nc


class _Runner:
    """Compile once, run many times: cached shard_map over the 8 cores."""

    def __init__(self, use_fp32r: bool, f32r_loads: bool = False):
        import jax
        import jax.core
        from jax.experimental.shard_map import shard_map
        from jax.sharding import Mesh, PartitionSpec

        import concourse.mybir as mybir
        from concourse import bass2jax

        self.nc = _build_nc(use_fp32r, f32r_loads)
        bass2jax.install_neuronx_cc_hook()

        partition_name = (
            self.nc.partition_id_tensor.name if self.nc.partition_id_tensor else None
        )
        in_names, out_names, out_avals = [], [], []
        for alloc in self.nc.m.functions[0].allocations:
            if not isinstance(alloc, mybir.MemoryLocationSet):
                continue
            name = alloc.memorylocations[0].name
            if alloc.kind == "ExternalInput":
                if name != partition_name:
                    in_names.append(name)
            elif alloc.kind == "ExternalOutput":
                out_names.append(name)
                out_avals.append(
                    jax.core.ShapedArray(
                        tuple(alloc.tensor_shape), mybir.dt.np(alloc.dtype)
                    )
                )
        self.in_names = in_names
        self.out_names = out_names
        self.out_shapes = [tuple(a.shape) for a in out_avals]
        self.out_dtypes = [a.dtype for a in out_avals]

        names_all = tuple(
            in_names + out_names + ([partition_name] if partition_name else [])
        )
        out_avals_t = tuple(out_avals)
        n_params = len(in_names)
        donate = tuple(range(n_params, n_params + len(out_names)))
        nc = self.nc

        def _body(*args):
            operands = list(args)
            if partition_name is not None:
                operands.append(bass2jax.partition_id_tensor())
            outs = bass2jax._bass_exec_p.bind(
                *operands,
                out_avals=out_avals_t,
                in_names=names_all,
                out_names=tuple(out_names),
                lowering_input_output_aliases=(),
                sim_require_finite=True,
                sim_require_nnan=True,
                nc=nc,
            )
            return tuple(outs)

        devices = jax.devices()[:NCORES]
        assert len(devices) == NCORES, f"need {NCORES} cores, saw {len(devices)}"
        mesh = Mesh(np.asarray(devices), ("core",))
        in_specs = (PartitionSpec("core"),) * (n_params + len(out_names))
        out_specs = (PartitionSpec("core"),) * len(out_names)
        self.fn = jax.jit(
            shard_map(
                _body,
                mesh=mesh,
                in_specs=in_specs,
                out_specs=out_specs,
                check_rep=False,
            ),
            donate_argnums=donate,
            keep_unused=True,
        )

    def __call__(self, in_maps):
        concat_in = [
            np.concatenate([np.asarray(m[name]) for m in in_maps], axis=0)
            for name in self.in_names
        ]
        concat_zeros = [
            np.zeros((NCORES * s[0], *s[1:]), d)
            for s, d in zip(self.out_shapes, self.out_dtypes)
        ]
        outs = self.fn(*concat_in, *concat_zeros)
        return [
            {
                name: np.asarray(outs[i]).reshape(NCORES, *self.out_shapes[i])[c]
                for i, name in enumerate(self.out_names)
            }
            for c in range(NCORES)
        ]


def _get_runner():
    key = (_use_fp32r(), _use_f32r_loads())
    if key not in _RUNNERS:
        _RUNNERS[key] = _Runner(*key)
    return _RUNNERS[key]


def make_in_maps(query, keys):
    """Shard full inputs batch-wise into the 8 per-core input dicts."""
    q = np.asarray(query, dtype=np.float32).reshape(L, BSZ, H)
    k = np.asarray(keys, dtype=np.float32).reshape(N, BSZ, L, H)
    in_maps = []
    for c in range(NCORES):
        sl = slice(c * BPC, (c + 1) * BPC)
        in_maps.append(
            {
                "q": np.ascontiguousarray(q[:, sl, :]),
                "k": np.ascontiguousarray(k[:, sl, :, :].transpose(1, 0, 2, 3)),
            }
        )
    return in_maps


def kernel(query, keys, values=None, **_unused):
    """Full-input entry point: returns (att (32,1,64) f32, topk_idx (8,32) i32)."""
    del values  # dead code in the reference module: never read
    runner = _get_runner()
    res = runner(make_in_maps(query, keys))
    att = np.concatenate([r["att"] for r in res], axis=0).reshape(BSZ, 1, N)
    idx = (
        np.concatenate([r["idx"] for r in res], axis=0)
        .astype(np.int32)
        .T.copy()
    )
    return att, idx


# revision 30
# speedup vs baseline: 1.0654x; 1.0654x over previous
"""Trainium2 Bass kernel for max-pooled KNN cache retrieval.

Computes, for each batch element b:
    att[b, n]   = max_{i,j} dot(query[b, i, :], keys[b, n, j, :])   (fp32)
    topk_idx    = indices of the 8 largest att[b, :] (descending)

Sharding: data-parallel over the batch dim (bsz=32 over 8 cores, 4 each).
The `values` input is dead code in the reference module and is never read.

Per-core pipeline (all under the Tile framework):
  1. DMA K rows naturally (nj on partitions, h in free dim, 1KB contiguous).
  2. PE-transpose 128x128 blocks so the contraction dim h lands on partitions.
  3. fp32r matmul  att_tile(64 i, 512 nj) = qT^T @ KT  (PSUM accumulate over h).
  4. DVE max-reduce over j per cache entry -> acc(64 i, 64 n) per batch.
  5. PE-transpose acc, DVE max-reduce over i -> att[b, n].
  6. vector.max / vector.max_index -> exact top-8 (desc, first-index ties).
"""

import os

import numpy as np

# Problem constants (hardcoded per harness contract).
L = 64          # num_steps (query rows i, key rows j per cache entry)
H = 256         # nhid (contraction dim)
N = 64          # cache entries
TOPK = 8
BSZ = 32
NCORES = 8
BPC = BSZ // NCORES   # batches per core = 4
NB = 8                # cache entries per chunk (= 512 K rows = one matmul tile)
NCHUNKS = N // NB     # 8 chunks per batch

_RUNNERS = {}


def _use_fp32r() -> bool:
    return os.environ.get("BASS_KNN_FP32R", "1") == "1"


def _use_f32r_loads() -> bool:
    return os.environ.get("BASS_KNN_F32R_LOADS", "0") == "1"


def _build_nc(use_fp32r: bool, f32r_loads: bool = False):
    import concourse.bacc as bacc
    import concourse.mybir as mybir
    from concourse.masks import make_identity
    from concourse.tile import TileContext

    f32 = mybir.dt.float32
    mm_dt = mybir.dt.float32r if use_fp32r else f32
    kd_dt = mm_dt if f32r_loads else f32

    nc = bacc.Bacc("TRN2", target_bir_lowering=False, debug=False)
    q_d = nc.declare_dram_parameter("q", [L, BPC, H], f32, isOutput=False)
    # per-core keys pre-permuted to [b, n, j, h] so (b, n) merges into a
    # single 128-partition outer DMA dim (full 16-engine fan-out).
    k_d = nc.declare_dram_parameter("k", [BPC, N, L, H], kd_dt, isOutput=False)
    att_d = nc.declare_dram_parameter("att", [BPC, N], f32, isOutput=True)
    idx_d = nc.declare_dram_parameter(
        "idx", [BPC, TOPK], mybir.dt.uint32, isOutput=True
    )

    AX = mybir.AxisListType
    MAX = mybir.AluOpType.max



    with TileContext(nc) as tc:
        with (
            tc.tile_pool(name="cpool", bufs=1) as cpool,
            tc.tile_pool(name="qpool", bufs=1) as qpool,
            tc.tile_pool(name="kpool", bufs=2) as kpool,
            tc.tile_pool(name="ktpool", bufs=1) as ktpool,
            tc.tile_pool(name="opool", bufs=1) as opool,
            tc.tile_pool(name="ps_kt", bufs=2, space="PSUM") as ps_kt,
            tc.tile_pool(name="ps_att", bufs=2, space="PSUM") as ps_att,
            tc.tile_pool(name="ps_sm", bufs=1, space="PSUM") as ps_sm,
        ):
            ident = cpool.tile([128, 128], f32)
            make_identity(nc, ident[:])
            if f32r_loads:
                identr_t = cpool.tile([128, 128], kd_dt, name="identr")
                nc.scalar.copy(identr_t[:], ident[:])
                identr = identr_t[:]
            else:
                identr = ident[:]

            # --- q prep: one DMA for all batches, then per-batch transposes
            qnat = qpool.tile([L, BPC * H], f32, name="qnat", tag="qnat")
            nc.sync.dma_start(out=qnat[:], in_=q_d[:, :, :])
            qTs = []
            for b in range(BPC):
                hs = []
                for hc in range(2):
                    pst = ps_sm.tile([128, L], f32, name="qt_ps", tag="qt_ps")
                    nc.tensor.transpose(
                        pst[:],
                        qnat[:, b * H + hc * 128 : b * H + (hc + 1) * 128],
                        ident[:L, :L],
                    )
                    qt = qpool.tile(
                        [128, L], mm_dt, name=f"qt{b}_{hc}", tag=f"qt{b}_{hc}"
                    )
                    nc.scalar.copy(qt[:], pst[:])
                    hs.append(qt)
                qTs.append(hs)

            # PE warmup: dense dummy matmuls while the first K DMA lands.
            # Keeps the PE clock gate (HAM) at full rate before the transpose
            # stream starts; output is never read.
            warm = ps_sm.tile([L, L], f32, name="warm", tag="qt_ps")
            for w in range(24):
                nc.tensor.matmul(
                    warm[:, :],
                    qTs[w % BPC][w % 2][:],
                    qTs[w % BPC][(w + 1) % 2][:],
                    start=True,
                    stop=True,
                )

            accs = [
                opool.tile([L, N], f32, name=f"acc{b}", tag=f"acc{b}")
                for b in range(BPC)
            ]

            # --- main loop: batches processed in pairs. K tile for a pair
            # and j-half: partition = 64*bb + n (bb = batch within pair),
            # free = j_local*256 + h -> each partition holds one entry's
            # contiguous 32KB j-half, so DMA descriptors are 32KB (vs 1KB
            # for an nj-partition layout). Two 2MB DMAs per tile, one per
            # batch band, issued on the two HWDGE rings.
            # PE transposes produce (h, nmix) columns; the PSUM->SBUF copy
            # re-strides them into KT tiles laid out n-major (col = nmix*32
            # + j_local) so matmul rhs slices are contiguous and the j-max
            # reduce keeps its one-op-per-chunk shape.
            JH = L // 2  # j rows per half-tile (32)
            for pair in range(2):
                kts = {}
                for jh in range(2):
                    ksb = kpool.tile([128, JH * H], kd_dt, name="ksb", tag="ksb")
                    # Two 2MB DMAs across all 128 partitions (16KB/partition
                    # descriptors, all 16 SDMA engines), one per HWDGE ring,
                    # split by j range so transposes start after the first.
                    for half, eng in ((0, nc.sync), (1, nc.scalar)):
                        j0 = jh * JH + half * (JH // 2)
                        eng.dma_start(
                            out=ksb[:, half * (JH // 2) * H : (half + 1) * (JH // 2) * H],
                            in_=k_d[
                                pair * 2 : pair * 2 + 2, :, j0 : j0 + JH // 2, :
                            ],
                        )
                    for hc in range(2):
                        kt = ktpool.tile(
                            [128, 128 * JH],
                            mm_dt,
                            name=f"kt{jh}_{hc}",
                            tag=f"kt{jh}_{hc}",
                            bufs=1,
                        )
                        ktv = kt.rearrange("p (n j) -> p n j", n=128)
                        for g in range(JH // 4):
                            ktp = ps_kt.tile(
                                [128, 512], kd_dt, name=f"ktp{hc}", tag=f"ktp{hc}"
                            )
                            for t in range(4):
                                jl = g * 4 + t
                                nc.tensor.transpose(
                                    ktp[:, t * 128 : (t + 1) * 128],
                                    ksb[
                                        :,
                                        jl * H
                                        + hc * 128 : jl * H
                                        + hc * 128
                                        + 128,
                                    ],
                                    identr,
                                )
                            if g % 4 == 3:
                                nc.vector.tensor_copy(
                                    out=ktv[:, :, g * 4 : (g + 1) * 4],
                                    in_=ktp[:].rearrange("p (j n) -> p n j", j=4),
                                )
                            else:
                                nc.scalar.copy(
                                    out=ktv[:, :, g * 4 : (g + 1) * 4],
                                    in_=ktp[:].rearrange("p (j n) -> p n j", j=4),
                                )
                        kts[(jh, hc)] = kt
                # matmuls: per batch, 8 chunks of 8 entries; attp columns =
                # (jh, n_local, j_local); one 4D reduce maxes over (jh, j).
                for bb in range(2):
                    b = pair * 2 + bb
                    for cx in range(8):
                        attp = ps_att.tile([L, 512], f32, name="attp", tag="attp")
                        for jh in range(2):
                            for hc in range(2):
                                nc.tensor.matmul(
                                    attp[:, jh * 256 : (jh + 1) * 256],
                                    qTs[b][hc][:],
                                    kts[(jh, hc)][
                                        :,
                                        bb * 64 * JH
                                        + cx * 256 : bb * 64 * JH
                                        + (cx + 1) * 256,
                                    ],
                                    start=(hc == 0),
                                    stop=(hc == 1),
                                )
                        nc.vector.tensor_reduce(
                            out=accs[b][:, cx * 8 : (cx + 1) * 8],
                            in_=attp[:].rearrange(
                                "p (jh n j) -> p n jh j", jh=2, n=8
                            ),
                            axis=AX.XY,
                            op=MAX,
                        )

            # --- epilogue: max over i, then top-8 per batch
            col = opool.tile([N, BPC], f32)
            for b in range(BPC):
                accT = ps_sm.tile([N, L], f32, name="accT", tag="accT")
                nc.tensor.transpose(accT[:], accs[b][:], ident[:L, :L])
                nc.vector.tensor_reduce(
                    out=col[:, b : b + 1], in_=accT[:], axis=AX.X, op=MAX
                )
            colT = ps_sm.tile([BPC, N], f32, name="colT", tag="accT")
            nc.tensor.transpose(colT[:], col[:], ident[:N, :N])
            attb = opool.tile([BPC, N], f32)
            nc.scalar.copy(attb[:], colT[:])
            mx = opool.tile([BPC, TOPK], f32)
            ix = opool.tile([BPC, TOPK], mybir.dt.uint32)
            nc.vector.max(out=mx[:], in_=attb[:])
            nc.vector.max_index(out=ix[:], in_max=mx[:], in_values=attb[:])
            nc.gpsimd.dma_start(out=att_d[:, :], in_=attb[:])
            nc.gpsimd.dma_start(out=idx_d[:, :], in_=ix[:])

    nc.finalize()
    return nc


class _Runner:
    """Compile once, run many times: cached shard_map over the 8 cores."""

    def __init__(self, use_fp32r: bool, f32r_loads: bool = False):
        import jax
        import jax.core
        from jax.experimental.shard_map import shard_map
        from jax.sharding import Mesh, PartitionSpec

        import concourse.mybir as mybir
        from concourse import bass2jax

        self.nc = _build_nc(use_fp32r, f32r_loads)
        bass2jax.install_neuronx_cc_hook()

        partition_name = (
            self.nc.partition_id_tensor.name if self.nc.partition_id_tensor else None
        )
        in_names, out_names, out_avals = [], [], []
        for alloc in self.nc.m.functions[0].allocations:
            if not isinstance(alloc, mybir.MemoryLocationSet):
                continue
            name = alloc.memorylocations[0].name
            if alloc.kind == "ExternalInput":
                if name != partition_name:
                    in_names.append(name)
            elif alloc.kind == "ExternalOutput":
                out_names.append(name)
                out_avals.append(
                    jax.core.ShapedArray(
                        tuple(alloc.tensor_shape), mybir.dt.np(alloc.dtype)
                    )
                )
        self.in_names = in_names
        self.out_names = out_names
        self.out_shapes = [tuple(a.shape) for a in out_avals]
        self.out_dtypes = [a.dtype for a in out_avals]

        names_all = tuple(
            in_names + out_names + ([partition_name] if partition_name else [])
        )
        out_avals_t = tuple(out_avals)
        n_params = len(in_names)
        donate = tuple(range(n_params, n_params + len(out_names)))
        nc = self.nc

        def _body(*args):
            operands = list(args)
            if partition_name is not None:
                operands.append(bass2jax.partition_id_tensor())
            outs = bass2jax._bass_exec_p.bind(
                *operands,
                out_avals=out_avals_t,
                in_names=names_all,
                out_names=tuple(out_names),
                lowering_input_output_aliases=(),
                sim_require_finite=True,
                sim_require_nnan=True,
                nc=nc,
            )
            return tuple(outs)

        devices = jax.devices()[:NCORES]
        assert len(devices) == NCORES, f"need {NCORES} cores, saw {len(devices)}"
        mesh = Mesh(np.asarray(devices), ("core",))
        in_specs = (PartitionSpec("core"),) * (n_params + len(out_names))
        out_specs = (PartitionSpec("core"),) * len(out_names)
        self.fn = jax.jit(
            shard_map(
                _body,
                mesh=mesh,
                in_specs=in_specs,
                out_specs=out_specs,
                check_rep=False,
            ),
            donate_argnums=donate,
            keep_unused=True,
        )

    def __call__(self, in_maps):
        concat_in = [
            np.concatenate([np.asarray(m[name]) for m in in_maps], axis=0)
            for name in self.in_names
        ]
        concat_zeros = [
            np.zeros((NCORES * s[0], *s[1:]), d)
            for s, d in zip(self.out_shapes, self.out_dtypes)
        ]
        outs = self.fn(*concat_in, *concat_zeros)
        return [
            {
                name: np.asarray(outs[i]).reshape(NCORES, *self.out_shapes[i])[c]
                for i, name in enumerate(self.out_names)
            }
            for c in range(NCORES)
        ]


def _get_runner():
    key = (_use_fp32r(), _use_f32r_loads())
    if key not in _RUNNERS:
        _RUNNERS[key] = _Runner(*key)
    return _RUNNERS[key]


def make_in_maps(query, keys):
    """Shard full inputs batch-wise into the 8 per-core input dicts."""
    q = np.asarray(query, dtype=np.float32).reshape(L, BSZ, H)
    k = np.asarray(keys, dtype=np.float32).reshape(N, BSZ, L, H)
    in_maps = []
    for c in range(NCORES):
        sl = slice(c * BPC, (c + 1) * BPC)
        in_maps.append(
            {
                "q": np.ascontiguousarray(q[:, sl, :]),
                "k": np.ascontiguousarray(k[:, sl, :, :].transpose(1, 0, 2, 3)),
            }
        )
    return in_maps


def kernel(query, keys, values=None, **_unused):
    """Full-input entry point: returns (att (32,1,64) f32, topk_idx (8,32) i32)."""
    del values  # dead code in the reference module: never read
    runner = _get_runner()
    res = runner(make_in_maps(query, keys))
    att = np.concatenate([r["att"] for r in res], axis=0).reshape(BSZ, 1, N)
    idx = (
        np.concatenate([r["idx"] for r in res], axis=0)
        .astype(np.int32)
        .T.copy()
    )
    return att, idx


# revision 31
# speedup vs baseline: 1.1275x; 1.0583x over previous
"""Trainium2 Bass kernel for max-pooled KNN cache retrieval.

Computes, for each batch element b:
    att[b, n]   = max_{i,j} dot(query[b, i, :], keys[b, n, j, :])   (fp32)
    topk_idx    = indices of the 8 largest att[b, :] (descending)

Sharding: data-parallel over the batch dim (bsz=32 over 8 cores, 4 each).
The `values` input is dead code in the reference module and is never read.

Per-core pipeline (all under the Tile framework):
  1. DMA K rows naturally (nj on partitions, h in free dim, 1KB contiguous).
  2. PE-transpose 128x128 blocks so the contraction dim h lands on partitions.
  3. fp32r matmul  att_tile(64 i, 512 nj) = qT^T @ KT  (PSUM accumulate over h).
  4. DVE max-reduce over j per cache entry -> acc(64 i, 64 n) per batch.
  5. PE-transpose acc, DVE max-reduce over i -> att[b, n].
  6. vector.max / vector.max_index -> exact top-8 (desc, first-index ties).
"""

import os

import numpy as np

# Problem constants (hardcoded per harness contract).
L = 64          # num_steps (query rows i, key rows j per cache entry)
H = 256         # nhid (contraction dim)
N = 64          # cache entries
TOPK = 8
BSZ = 32
NCORES = 8
BPC = BSZ // NCORES   # batches per core = 4
NB = 8                # cache entries per chunk (= 512 K rows = one matmul tile)
NCHUNKS = N // NB     # 8 chunks per batch

_RUNNERS = {}


def _use_fp32r() -> bool:
    return os.environ.get("BASS_KNN_FP32R", "1") == "1"


def _use_f32r_loads() -> bool:
    return os.environ.get("BASS_KNN_F32R_LOADS", "0") == "1"


def _build_nc(use_fp32r: bool, f32r_loads: bool = False):
    import concourse.bacc as bacc
    import concourse.mybir as mybir
    from concourse.masks import make_identity
    from concourse.tile import TileContext

    f32 = mybir.dt.float32
    mm_dt = mybir.dt.float32r if use_fp32r else f32
    kd_dt = mm_dt if f32r_loads else f32

    nc = bacc.Bacc("TRN2", target_bir_lowering=False, debug=False)
    q_d = nc.declare_dram_parameter("q", [L, BPC, H], f32, isOutput=False)
    # per-core keys pre-permuted to [b, n, j, h] so (b, n) merges into a
    # single 128-partition outer DMA dim (full 16-engine fan-out).
    k_d = nc.declare_dram_parameter("k", [BPC, N, L, H], kd_dt, isOutput=False)
    att_d = nc.declare_dram_parameter("att", [BPC, N], f32, isOutput=True)
    idx_d = nc.declare_dram_parameter(
        "idx", [BPC, TOPK], mybir.dt.uint32, isOutput=True
    )

    AX = mybir.AxisListType
    MAX = mybir.AluOpType.max



    with TileContext(nc) as tc:
        with (
            tc.tile_pool(name="cpool", bufs=1) as cpool,
            tc.tile_pool(name="qpool", bufs=1) as qpool,
            tc.tile_pool(name="kpool", bufs=2) as kpool,
            tc.tile_pool(name="ktpool", bufs=1) as ktpool,
            tc.tile_pool(name="opool", bufs=1) as opool,
            tc.tile_pool(name="ps_kt", bufs=2, space="PSUM") as ps_kt,
            tc.tile_pool(name="ps_att", bufs=2, space="PSUM") as ps_att,
            tc.tile_pool(name="ps_sm", bufs=1, space="PSUM") as ps_sm,
        ):
            ident = cpool.tile([128, 128], f32)
            make_identity(nc, ident[:])
            if f32r_loads:
                identr_t = cpool.tile([128, 128], kd_dt, name="identr")
                nc.scalar.copy(identr_t[:], ident[:])
                identr = identr_t[:]
            else:
                identr = ident[:]

            # --- q prep: one DMA for all batches, then per-batch transposes
            qnat = qpool.tile([L, BPC * H], f32, name="qnat", tag="qnat")
            nc.sync.dma_start(out=qnat[:], in_=q_d[:, :, :])
            qTs = []
            for b in range(BPC):
                hs = []
                for hc in range(2):
                    pst = ps_sm.tile([128, L], f32, name="qt_ps", tag="qt_ps")
                    nc.tensor.transpose(
                        pst[:],
                        qnat[:, b * H + hc * 128 : b * H + (hc + 1) * 128],
                        ident[:L, :L],
                    )
                    qt = qpool.tile(
                        [128, L], mm_dt, name=f"qt{b}_{hc}", tag=f"qt{b}_{hc}"
                    )
                    nc.scalar.copy(qt[:], pst[:])
                    hs.append(qt)
                qTs.append(hs)

            accs = [
                opool.tile([L, N], f32, name=f"acc{b}", tag=f"acc{b}")
                for b in range(BPC)
            ]

            # --- main loop: batches processed in pairs. K tile for a pair
            # and j-half: partition = 64*bb + n (bb = batch within pair),
            # free = j_local*256 + h -> each partition holds one entry's
            # contiguous 32KB j-half, so DMA descriptors are 32KB (vs 1KB
            # for an nj-partition layout). Two 2MB DMAs per tile, one per
            # batch band, issued on the two HWDGE rings.
            # PE transposes produce (h, nmix) columns; the PSUM->SBUF copy
            # re-strides them into KT tiles laid out n-major (col = nmix*32
            # + j_local) so matmul rhs slices are contiguous and the j-max
            # reduce keeps its one-op-per-chunk shape.
            JH = L // 2  # j rows per half-tile (32)
            for pair in range(2):
                kts = {}
                for jh in range(2):
                    ksb = kpool.tile([128, JH * H], kd_dt, name="ksb", tag="ksb")
                    # Four 1MB DMAs across all 128 partitions (8KB/partition
                    # descriptors, all 16 SDMA engines), alternating the two
                    # HWDGE rings, split by j range so the first transposes
                    # start after ~1MB instead of a whole tile.
                    JQ = JH // 4
                    for quart in range(4):
                        eng = nc.sync if quart % 2 == 0 else nc.scalar
                        j0 = jh * JH + quart * JQ
                        eng.dma_start(
                            out=ksb[:, quart * JQ * H : (quart + 1) * JQ * H],
                            in_=k_d[pair * 2 : pair * 2 + 2, :, j0 : j0 + JQ, :],
                        )
                    for hc in range(2):
                        kt = ktpool.tile(
                            [128, 128 * JH],
                            mm_dt,
                            name=f"kt{jh}_{hc}",
                            tag=f"kt{jh}_{hc}",
                            bufs=2,
                        )
                        ktv = kt.rearrange("p (n j) -> p n j", n=128)
                        for g in range(JH // 4):
                            ktp = ps_kt.tile(
                                [128, 512], kd_dt, name=f"ktp{hc}", tag=f"ktp{hc}"
                            )
                            for t in range(4):
                                jl = g * 4 + t
                                nc.tensor.transpose(
                                    ktp[:, t * 128 : (t + 1) * 128],
                                    ksb[
                                        :,
                                        jl * H
                                        + hc * 128 : jl * H
                                        + hc * 128
                                        + 128,
                                    ],
                                    identr,
                                )
                            if g % 4 == 3:
                                nc.vector.tensor_copy(
                                    out=ktv[:, :, g * 4 : (g + 1) * 4],
                                    in_=ktp[:].rearrange("p (j n) -> p n j", j=4),
                                )
                            else:
                                nc.scalar.copy(
                                    out=ktv[:, :, g * 4 : (g + 1) * 4],
                                    in_=ktp[:].rearrange("p (j n) -> p n j", j=4),
                                )
                        kts[(jh, hc)] = kt
                # matmuls: per batch, 8 chunks of 8 entries; attp columns =
                # (jh, n_local, j_local); one 4D reduce maxes over (jh, j).
                for bb in range(2):
                    b = pair * 2 + bb
                    for cx in range(8):
                        attp = ps_att.tile([L, 512], f32, name="attp", tag="attp")
                        for jh in range(2):
                            for hc in range(2):
                                nc.tensor.matmul(
                                    attp[:, jh * 256 : (jh + 1) * 256],
                                    qTs[b][hc][:],
                                    kts[(jh, hc)][
                                        :,
                                        bb * 64 * JH
                                        + cx * 256 : bb * 64 * JH
                                        + (cx + 1) * 256,
                                    ],
                                    start=(hc == 0),
                                    stop=(hc == 1),
                                )
                        nc.vector.tensor_reduce(
                            out=accs[b][:, cx * 8 : (cx + 1) * 8],
                            in_=attp[:].rearrange(
                                "p (jh n j) -> p n jh j", jh=2, n=8
                            ),
                            axis=AX.XY,
                            op=MAX,
                        )

            # --- epilogue: max over i, then top-8 per batch
            col = opool.tile([N, BPC], f32)
            for b in range(BPC):
                accT = ps_sm.tile([N, L], f32, name="accT", tag="accT")
                nc.tensor.transpose(accT[:], accs[b][:], ident[:L, :L])
                nc.vector.tensor_reduce(
                    out=col[:, b : b + 1], in_=accT[:], axis=AX.X, op=MAX
                )
            colT = ps_sm.tile([BPC, N], f32, name="colT", tag="accT")
            nc.tensor.transpose(colT[:], col[:], ident[:N, :N])
            attb = opool.tile([BPC, N], f32)
            nc.scalar.copy(attb[:], colT[:])
            mx = opool.tile([BPC, TOPK], f32)
            ix = opool.tile([BPC, TOPK], mybir.dt.uint32)
            nc.vector.max(out=mx[:], in_=attb[:])
            nc.vector.max_index(out=ix[:], in_max=mx[:], in_values=attb[:])
            nc.gpsimd.dma_start(out=att_d[:, :], in_=attb[:])
            nc.gpsimd.dma_start(out=idx_d[:, :], in_=ix[:])

    nc.finalize()
    return nc


class _Runner:
    """Compile once, run many times: cached shard_map over the 8 cores."""

    def __init__(self, use_fp32r: bool, f32r_loads: bool = False):
        import jax
        import jax.core
        from jax.experimental.shard_map import shard_map
        from jax.sharding import Mesh, PartitionSpec

        import concourse.mybir as mybir
        from concourse import bass2jax

        self.nc = _build_nc(use_fp32r, f32r_loads)
        bass2jax.install_neuronx_cc_hook()

        partition_name = (
            self.nc.partition_id_tensor.name if self.nc.partition_id_tensor else None
        )
        in_names, out_names, out_avals = [], [], []
        for alloc in self.nc.m.functions[0].allocations:
            if not isinstance(alloc, mybir.MemoryLocationSet):
                continue
            name = alloc.memorylocations[0].name
            if alloc.kind == "ExternalInput":
                if name != partition_name:
                    in_names.append(name)
            elif alloc.kind == "ExternalOutput":
                out_names.append(name)
                out_avals.append(
                    jax.core.ShapedArray(
                        tuple(alloc.tensor_shape), mybir.dt.np(alloc.dtype)
                    )
                )
        self.in_names = in_names
        self.out_names = out_names
        self.out_shapes = [tuple(a.shape) for a in out_avals]
        self.out_dtypes = [a.dtype for a in out_avals]

        names_all = tuple(
            in_names + out_names + ([partition_name] if partition_name else [])
        )
        out_avals_t = tuple(out_avals)
        n_params = len(in_names)
        donate = tuple(range(n_params, n_params + len(out_names)))
        nc = self.nc

        def _body(*args):
            operands = list(args)
            if partition_name is not None:
                operands.append(bass2jax.partition_id_tensor())
            outs = bass2jax._bass_exec_p.bind(
                *operands,
                out_avals=out_avals_t,
                in_names=names_all,
                out_names=tuple(out_names),
                lowering_input_output_aliases=(),
                sim_require_finite=True,
                sim_require_nnan=True,
                nc=nc,
            )
            return tuple(outs)

        devices = jax.devices()[:NCORES]
        assert len(devices) == NCORES, f"need {NCORES} cores, saw {len(devices)}"
        mesh = Mesh(np.asarray(devices), ("core",))
        in_specs = (PartitionSpec("core"),) * (n_params + len(out_names))
        out_specs = (PartitionSpec("core"),) * len(out_names)
        self.fn = jax.jit(
            shard_map(
                _body,
                mesh=mesh,
                in_specs=in_specs,
                out_specs=out_specs,
                check_rep=False,
            ),
            donate_argnums=donate,
            keep_unused=True,
        )

    def __call__(self, in_maps):
        concat_in = [
            np.concatenate([np.asarray(m[name]) for m in in_maps], axis=0)
            for name in self.in_names
        ]
        concat_zeros = [
            np.zeros((NCORES * s[0], *s[1:]), d)
            for s, d in zip(self.out_shapes, self.out_dtypes)
        ]
        outs = self.fn(*concat_in, *concat_zeros)
        return [
            {
                name: np.asarray(outs[i]).reshape(NCORES, *self.out_shapes[i])[c]
                for i, name in enumerate(self.out_names)
            }
            for c in range(NCORES)
        ]


def _get_runner():
    key = (_use_fp32r(), _use_f32r_loads())
    if key not in _RUNNERS:
        _RUNNERS[key] = _Runner(*key)
    return _RUNNERS[key]


def make_in_maps(query, keys):
    """Shard full inputs batch-wise into the 8 per-core input dicts."""
    q = np.asarray(query, dtype=np.float32).reshape(L, BSZ, H)
    k = np.asarray(keys, dtype=np.float32).reshape(N, BSZ, L, H)
    in_maps = []
    for c in range(NCORES):
        sl = slice(c * BPC, (c + 1) * BPC)
        in_maps.append(
            {
                "q": np.ascontiguousarray(q[:, sl, :]),
                "k": np.ascontiguousarray(k[:, sl, :, :].transpose(1, 0, 2, 3)),
            }
        )
    return in_maps


def kernel(query, keys, values=None, **_unused):
    """Full-input entry point: returns (att (32,1,64) f32, topk_idx (8,32) i32)."""
    del values  # dead code in the reference module: never read
    runner = _get_runner()
    res = runner(make_in_maps(query, keys))
    att = np.concatenate([r["att"] for r in res], axis=0).reshape(BSZ, 1, N)
    idx = (
        np.concatenate([r["idx"] for r in res], axis=0)
        .astype(np.int32)
        .T.copy()
    )
    return att, idx


# revision 32
# speedup vs baseline: 1.2482x; 1.1071x over previous
"""Trainium2 Bass kernel for max-pooled KNN cache retrieval.

Computes, for each batch element b:
    att[b, n]   = max_{i,j} dot(query[b, i, :], keys[b, n, j, :])   (fp32)
    topk_idx    = indices of the 8 largest att[b, :] (descending)

Sharding: data-parallel over the batch dim (bsz=32 over 8 cores, 4 each).
The `values` input is dead code in the reference module and is never read.

Per-core pipeline (all under the Tile framework):
  1. DMA K rows naturally (nj on partitions, h in free dim, 1KB contiguous).
  2. PE-transpose 128x128 blocks so the contraction dim h lands on partitions.
  3. fp32r matmul  att_tile(64 i, 512 nj) = qT^T @ KT  (PSUM accumulate over h).
  4. DVE max-reduce over j per cache entry -> acc(64 i, 64 n) per batch.
  5. PE-transpose acc, DVE max-reduce over i -> att[b, n].
  6. vector.max / vector.max_index -> exact top-8 (desc, first-index ties).
"""

import os

import numpy as np

# Problem constants (hardcoded per harness contract).
L = 64          # num_steps (query rows i, key rows j per cache entry)
H = 256         # nhid (contraction dim)
N = 64          # cache entries
TOPK = 8
BSZ = 32
NCORES = 8
BPC = BSZ // NCORES   # batches per core = 4
NB = 8                # cache entries per chunk (= 512 K rows = one matmul tile)
NCHUNKS = N // NB     # 8 chunks per batch

_RUNNERS = {}


def _use_fp32r() -> bool:
    return os.environ.get("BASS_KNN_FP32R", "1") == "1"


def _use_f32r_loads() -> bool:
    return os.environ.get("BASS_KNN_F32R_LOADS", "0") == "1"


def _build_nc(use_fp32r: bool, f32r_loads: bool = False):
    import concourse.bacc as bacc
    import concourse.mybir as mybir
    from concourse.masks import make_identity
    from concourse.tile import TileContext

    f32 = mybir.dt.float32
    mm_dt = mybir.dt.float32r if use_fp32r else f32
    kd_dt = mm_dt if f32r_loads else f32

    nc = bacc.Bacc("TRN2", target_bir_lowering=False, debug=False)
    q_d = nc.declare_dram_parameter("q", [L, BPC, H], f32, isOutput=False)
    # per-core keys pre-permuted to [b, n, j, h] so (b, n) merges into a
    # single 128-partition outer DMA dim (full 16-engine fan-out).
    k_d = nc.declare_dram_parameter("k", [BPC, N, L, H], kd_dt, isOutput=False)
    att_d = nc.declare_dram_parameter("att", [BPC, N], f32, isOutput=True)
    idx_d = nc.declare_dram_parameter(
        "idx", [BPC, TOPK], mybir.dt.uint32, isOutput=True
    )

    AX = mybir.AxisListType
    MAX = mybir.AluOpType.max



    with TileContext(nc) as tc:
        with (
            tc.tile_pool(name="cpool", bufs=1) as cpool,
            tc.tile_pool(name="qpool", bufs=1) as qpool,
            tc.tile_pool(name="kpool", bufs=2) as kpool,
            tc.tile_pool(name="ktpool", bufs=1) as ktpool,
            tc.tile_pool(name="opool", bufs=1) as opool,
            tc.tile_pool(name="ps_kt", bufs=2, space="PSUM") as ps_kt,
            tc.tile_pool(name="ps_att", bufs=2, space="PSUM") as ps_att,
            tc.tile_pool(name="ps_sm", bufs=1, space="PSUM") as ps_sm,
        ):
            ident = cpool.tile([128, 128], f32)
            make_identity(nc, ident[:])
            if f32r_loads:
                identr_t = cpool.tile([128, 128], kd_dt, name="identr")
                nc.scalar.copy(identr_t[:], ident[:])
                identr = identr_t[:]
            else:
                identr = ident[:]

            # --- q prep: one DMA for all batches, then per-batch transposes
            qnat = qpool.tile([L, BPC * H], f32, name="qnat", tag="qnat")
            nc.sync.dma_start(out=qnat[:], in_=q_d[:, :, :])
            qTs = []
            for b in range(BPC):
                hs = []
                for hc in range(2):
                    pst = ps_sm.tile([128, L], f32, name="qt_ps", tag="qt_ps")
                    nc.tensor.transpose(
                        pst[:],
                        qnat[:, b * H + hc * 128 : b * H + (hc + 1) * 128],
                        ident[:L, :L],
                    )
                    qt = qpool.tile(
                        [128, L], mm_dt, name=f"qt{b}_{hc}", tag=f"qt{b}_{hc}"
                    )
                    nc.scalar.copy(qt[:], pst[:])
                    hs.append(qt)
                qTs.append(hs)

            # PE warmup: dense dummy matmuls fill the PE while the first K
            # DMA lands and flip the HAM clock gate to full rate. An explicit
            # ordering edge onto the first K transpose keeps the scheduler
            # from deferring them (their output is never read).
            warm = ps_sm.tile([L, L], f32, name="warm", tag="qt_ps")
            warm_insts = []
            for w in range(24):
                warm_insts.append(
                    nc.tensor.matmul(
                        warm[:, :],
                        qTs[w % BPC][w % 2][:],
                        qTs[w % BPC][(w + 1) % 2][:],
                        start=True,
                        stop=True,
                    )
                )
            first_k_transpose = [None]

            accs = [
                opool.tile([L, N], f32, name=f"acc{b}", tag=f"acc{b}")
                for b in range(BPC)
            ]

            # --- main loop: batches processed in pairs. K tile for a pair
            # and j-half: partition = 64*bb + n (bb = batch within pair),
            # free = j_local*256 + h -> each partition holds one entry's
            # contiguous 32KB j-half, so DMA descriptors are 32KB (vs 1KB
            # for an nj-partition layout). Two 2MB DMAs per tile, one per
            # batch band, issued on the two HWDGE rings.
            # PE transposes produce (h, nmix) columns; the PSUM->SBUF copy
            # re-strides them into KT tiles laid out n-major (col = nmix*32
            # + j_local) so matmul rhs slices are contiguous and the j-max
            # reduce keeps its one-op-per-chunk shape.
            JH = L // 2  # j rows per half-tile (32)
            for pair in range(2):
                kts = {}
                for jh in range(2):
                    ksb = kpool.tile([128, JH * H], kd_dt, name="ksb", tag="ksb")
                    # Four 1MB DMAs across all 128 partitions (8KB/partition
                    # descriptors, all 16 SDMA engines), alternating the two
                    # HWDGE rings, split by j range so the first transposes
                    # start after ~1MB instead of a whole tile.
                    nsplit = 8 if (pair == 0 and jh == 0) else 4
                    JQ = JH // nsplit
                    for quart in range(nsplit):
                        eng = nc.sync if quart % 2 == 0 else nc.scalar
                        j0 = jh * JH + quart * JQ
                        eng.dma_start(
                            out=ksb[:, quart * JQ * H : (quart + 1) * JQ * H],
                            in_=k_d[pair * 2 : pair * 2 + 2, :, j0 : j0 + JQ, :],
                        )
                    for hc in range(2):
                        kt = ktpool.tile(
                            [128, 128 * JH],
                            mm_dt,
                            name=f"kt{jh}_{hc}",
                            tag=f"kt{jh}_{hc}",
                            bufs=2,
                        )
                        ktv = kt.rearrange("p (n j) -> p n j", n=128)
                        for g in range(JH // 4):
                            ktp = ps_kt.tile(
                                [128, 512], kd_dt, name=f"ktp{hc}", tag=f"ktp{hc}"
                            )
                            for t in range(4):
                                jl = g * 4 + t
                                tri = nc.tensor.transpose(
                                    ktp[:, t * 128 : (t + 1) * 128],
                                    ksb[
                                        :,
                                        jl * H
                                        + hc * 128 : jl * H
                                        + hc * 128
                                        + 128,
                                    ],
                                    identr,
                                )
                                if first_k_transpose[0] is None:
                                    first_k_transpose[0] = tri
                                    from concourse.bass import _add_dep_helper

                                    _add_dep_helper(
                                        tri.ins,
                                        warm_insts[-1].ins,
                                        sync=False,
                                        reason="PE warmup before transposes",
                                    )
                            if g % 4 == 3:
                                nc.vector.tensor_copy(
                                    out=ktv[:, :, g * 4 : (g + 1) * 4],
                                    in_=ktp[:].rearrange("p (j n) -> p n j", j=4),
                                )
                            else:
                                nc.scalar.copy(
                                    out=ktv[:, :, g * 4 : (g + 1) * 4],
                                    in_=ktp[:].rearrange("p (j n) -> p n j", j=4),
                                )
                        kts[(jh, hc)] = kt
                # matmuls: per batch, 8 chunks of 8 entries; attp columns =
                # (jh, n_local, j_local); one 4D reduce maxes over (jh, j).
                for bb in range(2):
                    b = pair * 2 + bb
                    for cx in range(8):
                        attp = ps_att.tile([L, 512], f32, name="attp", tag="attp")
                        for jh in range(2):
                            for hc in range(2):
                                nc.tensor.matmul(
                                    attp[:, jh * 256 : (jh + 1) * 256],
                                    qTs[b][hc][:],
                                    kts[(jh, hc)][
                                        :,
                                        bb * 64 * JH
                                        + cx * 256 : bb * 64 * JH
                                        + (cx + 1) * 256,
                                    ],
                                    start=(hc == 0),
                                    stop=(hc == 1),
                                )
                        nc.vector.tensor_reduce(
                            out=accs[b][:, cx * 8 : (cx + 1) * 8],
                            in_=attp[:].rearrange(
                                "p (jh n j) -> p n jh j", jh=2, n=8
                            ),
                            axis=AX.XY,
                            op=MAX,
                        )

            # --- epilogue: max over i, then top-8 per batch
            col = opool.tile([N, BPC], f32)
            for b in range(BPC):
                accT = ps_sm.tile([N, L], f32, name="accT", tag="accT")
                nc.tensor.transpose(accT[:], accs[b][:], ident[:L, :L])
                nc.vector.tensor_reduce(
                    out=col[:, b : b + 1], in_=accT[:], axis=AX.X, op=MAX
                )
            colT = ps_sm.tile([BPC, N], f32, name="colT", tag="accT")
            nc.tensor.transpose(colT[:], col[:], ident[:N, :N])
            attb = opool.tile([BPC, N], f32)
            nc.scalar.copy(attb[:], colT[:])
            mx = opool.tile([BPC, TOPK], f32)
            ix = opool.tile([BPC, TOPK], mybir.dt.uint32)
            nc.vector.max(out=mx[:], in_=attb[:])
            nc.vector.max_index(out=ix[:], in_max=mx[:], in_values=attb[:])
            nc.gpsimd.dma_start(out=att_d[:, :], in_=attb[:])
            nc.gpsimd.dma_start(out=idx_d[:, :], in_=ix[:])

    nc.finalize()
    return nc


class _Runner:
    """Compile once, run many times: cached shard_map over the 8 cores."""

    def __init__(self, use_fp32r: bool, f32r_loads: bool = False):
        import jax
        import jax.core
        from jax.experimental.shard_map import shard_map
        from jax.sharding import Mesh, PartitionSpec

        import concourse.mybir as mybir
        from concourse import bass2jax

        self.nc = _build_nc(use_fp32r, f32r_loads)
        bass2jax.install_neuronx_cc_hook()

        partition_name = (
            self.nc.partition_id_tensor.name if self.nc.partition_id_tensor else None
        )
        in_names, out_names, out_avals = [], [], []
        for alloc in self.nc.m.functions[0].allocations:
            if not isinstance(alloc, mybir.MemoryLocationSet):
                continue
            name = alloc.memorylocations[0].name
            if alloc.kind == "ExternalInput":
                if name != partition_name:
                    in_names.append(name)
            elif alloc.kind == "ExternalOutput":
                out_names.append(name)
                out_avals.append(
                    jax.core.ShapedArray(
                        tuple(alloc.tensor_shape), mybir.dt.np(alloc.dtype)
                    )
                )
        self.in_names = in_names
        self.out_names = out_names
        self.out_shapes = [tuple(a.shape) for a in out_avals]
        self.out_dtypes = [a.dtype for a in out_avals]

        names_all = tuple(
            in_names + out_names + ([partition_name] if partition_name else [])
        )
        out_avals_t = tuple(out_avals)
        n_params = len(in_names)
        donate = tuple(range(n_params, n_params + len(out_names)))
        nc = self.nc

        def _body(*args):
            operands = list(args)
            if partition_name is not None:
                operands.append(bass2jax.partition_id_tensor())
            outs = bass2jax._bass_exec_p.bind(
                *operands,
                out_avals=out_avals_t,
                in_names=names_all,
                out_names=tuple(out_names),
                lowering_input_output_aliases=(),
                sim_require_finite=True,
                sim_require_nnan=True,
                nc=nc,
            )
            return tuple(outs)

        devices = jax.devices()[:NCORES]
        assert len(devices) == NCORES, f"need {NCORES} cores, saw {len(devices)}"
        mesh = Mesh(np.asarray(devices), ("core",))
        in_specs = (PartitionSpec("core"),) * (n_params + len(out_names))
        out_specs = (PartitionSpec("core"),) * len(out_names)
        self.fn = jax.jit(
            shard_map(
                _body,
                mesh=mesh,
                in_specs=in_specs,
                out_specs=out_specs,
                check_rep=False,
            ),
            donate_argnums=donate,
            keep_unused=True,
        )

    def __call__(self, in_maps):
        concat_in = [
            np.concatenate([np.asarray(m[name]) for m in in_maps], axis=0)
            for name in self.in_names
        ]
        concat_zeros = [
            np.zeros((NCORES * s[0], *s[1:]), d)
            for s, d in zip(self.out_shapes, self.out_dtypes)
        ]
        outs = self.fn(*concat_in, *concat_zeros)
        return [
            {
                name: np.asarray(outs[i]).reshape(NCORES, *self.out_shapes[i])[c]
                for i, name in enumerate(self.out_names)
            }
            for c in range(NCORES)
        ]


def _get_runner():
    key = (_use_fp32r(), _use_f32r_loads())
    if key not in _RUNNERS:
        _RUNNERS[key] = _Runner(*key)
    return _RUNNERS[key]


def make_in_maps(query, keys):
    """Shard full inputs batch-wise into the 8 per-core input dicts."""
    q = np.asarray(query, dtype=np.float32).reshape(L, BSZ, H)
    k = np.asarray(keys, dtype=np.float32).reshape(N, BSZ, L, H)
    in_maps = []
    for c in range(NCORES):
        sl = slice(c * BPC, (c + 1) * BPC)
        in_maps.append(
            {
                "q": np.ascontiguousarray(q[:, sl, :]),
                "k": np.ascontiguousarray(k[:, sl, :, :].transpose(1, 0, 2, 3)),
            }
        )
    return in_maps


def kernel(query, keys, values=None, **_unused):
    """Full-input entry point: returns (att (32,1,64) f32, topk_idx (8,32) i32)."""
    del values  # dead code in the reference module: never read
    runner = _get_runner()
    res = runner(make_in_maps(query, keys))
    att = np.concatenate([r["att"] for r in res], axis=0).reshape(BSZ, 1, N)
    idx = (
        np.concatenate([r["idx"] for r in res], axis=0)
        .astype(np.int32)
        .T.copy()
    )
    return att, idx


# revision 33
# speedup vs baseline: 1.2664x; 1.0145x over previous
"""Trainium2 Bass kernel for max-pooled KNN cache retrieval.

Computes, for each batch element b:
    att[b, n]   = max_{i,j} dot(query[b, i, :], keys[b, n, j, :])   (fp32)
    topk_idx    = indices of the 8 largest att[b, :] (descending)

Sharding: data-parallel over the batch dim (bsz=32 over 8 cores, 4 each).
The `values` input is dead code in the reference module and is never read.

Per-core pipeline (all under the Tile framework):
  1. DMA K rows naturally (nj on partitions, h in free dim, 1KB contiguous).
  2. PE-transpose 128x128 blocks so the contraction dim h lands on partitions.
  3. fp32r matmul  att_tile(64 i, 512 nj) = qT^T @ KT  (PSUM accumulate over h).
  4. DVE max-reduce over j per cache entry -> acc(64 i, 64 n) per batch.
  5. PE-transpose acc, DVE max-reduce over i -> att[b, n].
  6. vector.max / vector.max_index -> exact top-8 (desc, first-index ties).
"""

import os

import numpy as np

# Problem constants (hardcoded per harness contract).
L = 64          # num_steps (query rows i, key rows j per cache entry)
H = 256         # nhid (contraction dim)
N = 64          # cache entries
TOPK = 8
BSZ = 32
NCORES = 8
BPC = BSZ // NCORES   # batches per core = 4
NB = 8                # cache entries per chunk (= 512 K rows = one matmul tile)
NCHUNKS = N // NB     # 8 chunks per batch

_RUNNERS = {}


def _use_fp32r() -> bool:
    return os.environ.get("BASS_KNN_FP32R", "1") == "1"


def _use_f32r_loads() -> bool:
    return os.environ.get("BASS_KNN_F32R_LOADS", "0") == "1"


def _build_nc(use_fp32r: bool, f32r_loads: bool = False):
    import concourse.bacc as bacc
    import concourse.mybir as mybir
    from concourse.masks import make_identity
    from concourse.tile import TileContext

    f32 = mybir.dt.float32
    mm_dt = mybir.dt.float32r if use_fp32r else f32
    kd_dt = mm_dt if f32r_loads else f32

    nc = bacc.Bacc("TRN2", target_bir_lowering=False, debug=False)
    q_d = nc.declare_dram_parameter("q", [L, BPC, H], f32, isOutput=False)
    # per-core keys pre-permuted to [b, n, j, h] so (b, n) merges into a
    # single 128-partition outer DMA dim (full 16-engine fan-out).
    k_d = nc.declare_dram_parameter("k", [BPC, N, L, H], kd_dt, isOutput=False)
    att_d = nc.declare_dram_parameter("att", [BPC, N], f32, isOutput=True)
    idx_d = nc.declare_dram_parameter(
        "idx", [BPC, TOPK], mybir.dt.uint32, isOutput=True
    )

    AX = mybir.AxisListType
    MAX = mybir.AluOpType.max



    with TileContext(nc) as tc:
        with (
            tc.tile_pool(name="cpool", bufs=1) as cpool,
            tc.tile_pool(name="qpool", bufs=1) as qpool,
            tc.tile_pool(name="kpool", bufs=2) as kpool,
            tc.tile_pool(name="ktpool", bufs=1) as ktpool,
            tc.tile_pool(name="opool", bufs=1) as opool,
            tc.tile_pool(name="ps_kt", bufs=2, space="PSUM") as ps_kt,
            tc.tile_pool(name="ps_att", bufs=3, space="PSUM") as ps_att,
            tc.tile_pool(name="ps_sm", bufs=1, space="PSUM") as ps_sm,
        ):
            ident = cpool.tile([128, 128], f32)
            make_identity(nc, ident[:])
            if f32r_loads:
                identr_t = cpool.tile([128, 128], kd_dt, name="identr")
                nc.scalar.copy(identr_t[:], ident[:])
                identr = identr_t[:]
            else:
                identr = ident[:]

            # --- q prep: one DMA for all batches, then per-batch transposes
            qnat = qpool.tile([L, BPC * H], f32, name="qnat", tag="qnat")
            nc.sync.dma_start(out=qnat[:], in_=q_d[:, :, :])
            qTs = []
            for b in range(BPC):
                hs = []
                for hc in range(2):
                    pst = ps_sm.tile([128, L], f32, name="qt_ps", tag="qt_ps")
                    nc.tensor.transpose(
                        pst[:],
                        qnat[:, b * H + hc * 128 : b * H + (hc + 1) * 128],
                        ident[:L, :L],
                    )
                    qt = qpool.tile(
                        [128, L], mm_dt, name=f"qt{b}_{hc}", tag=f"qt{b}_{hc}"
                    )
                    nc.scalar.copy(qt[:], pst[:])
                    hs.append(qt)
                qTs.append(hs)

            # PE warmup: dense dummy matmuls fill the PE while the first K
            # DMA lands and flip the HAM clock gate to full rate. An explicit
            # ordering edge onto the first K transpose keeps the scheduler
            # from deferring them (their output is never read).
            warm = ps_sm.tile([L, L], f32, name="warm", tag="qt_ps")
            warm_insts = []
            for w in range(48):
                warm_insts.append(
                    nc.tensor.matmul(
                        warm[:, :],
                        qTs[w % BPC][w % 2][:],
                        qTs[w % BPC][(w + 1) % 2][:],
                        start=True,
                        stop=True,
                    )
                )
            first_k_transpose = [None]

            accs = [
                opool.tile([L, N], f32, name=f"acc{b}", tag=f"acc{b}")
                for b in range(BPC)
            ]

            # --- main loop: batches processed in pairs. K tile for a pair
            # and j-half: partition = 64*bb + n (bb = batch within pair),
            # free = j_local*256 + h -> each partition holds one entry's
            # contiguous 32KB j-half, so DMA descriptors are 32KB (vs 1KB
            # for an nj-partition layout). Two 2MB DMAs per tile, one per
            # batch band, issued on the two HWDGE rings.
            # PE transposes produce (h, nmix) columns; the PSUM->SBUF copy
            # re-strides them into KT tiles laid out n-major (col = nmix*32
            # + j_local) so matmul rhs slices are contiguous and the j-max
            # reduce keeps its one-op-per-chunk shape.
            JH = L // 2  # j rows per half-tile (32)
            for pair in range(2):
                kts = {}
                for jh in range(2):
                    ksb = kpool.tile([128, JH * H], kd_dt, name="ksb", tag="ksb")
                    # Four 1MB DMAs across all 128 partitions (8KB/partition
                    # descriptors, all 16 SDMA engines), alternating the two
                    # HWDGE rings, split by j range so the first transposes
                    # start after ~1MB instead of a whole tile.
                    nsplit = 8 if (pair == 0 and jh == 0) else 4
                    JQ = JH // nsplit
                    for quart in range(nsplit):
                        eng = nc.sync if quart % 2 == 0 else nc.scalar
                        j0 = jh * JH + quart * JQ
                        eng.dma_start(
                            out=ksb[:, quart * JQ * H : (quart + 1) * JQ * H],
                            in_=k_d[pair * 2 : pair * 2 + 2, :, j0 : j0 + JQ, :],
                        )
                    for hc in range(2):
                        kt = ktpool.tile(
                            [128, 128 * JH],
                            mm_dt,
                            name=f"kt{jh}_{hc}",
                            tag=f"kt{jh}_{hc}",
                            bufs=2,
                        )
                        ktv = kt.rearrange("p (n j) -> p n j", n=128)
                        for g in range(JH // 4):
                            ktp = ps_kt.tile(
                                [128, 512], kd_dt, name=f"ktp{hc}", tag=f"ktp{hc}"
                            )
                            for t in range(4):
                                jl = g * 4 + t
                                tri = nc.tensor.transpose(
                                    ktp[:, t * 128 : (t + 1) * 128],
                                    ksb[
                                        :,
                                        jl * H
                                        + hc * 128 : jl * H
                                        + hc * 128
                                        + 128,
                                    ],
                                    identr,
                                )
                                if first_k_transpose[0] is None:
                                    first_k_transpose[0] = tri
                                    from concourse.bass import _add_dep_helper

                                    _add_dep_helper(
                                        tri.ins,
                                        warm_insts[-1].ins,
                                        sync=False,
                                        reason="PE warmup before transposes",
                                    )
                            if g % 4 == 3:
                                nc.vector.tensor_copy(
                                    out=ktv[:, :, g * 4 : (g + 1) * 4],
                                    in_=ktp[:].rearrange("p (j n) -> p n j", j=4),
                                )
                            else:
                                nc.scalar.copy(
                                    out=ktv[:, :, g * 4 : (g + 1) * 4],
                                    in_=ktp[:].rearrange("p (j n) -> p n j", j=4),
                                )
                        kts[(jh, hc)] = kt
                # matmuls: per batch, 8 chunks of 8 entries; attp columns =
                # (jh, n_local, j_local); one 4D reduce maxes over (jh, j).
                for bb in range(2):
                    b = pair * 2 + bb
                    for cx in range(8):
                        attp = ps_att.tile([L, 512], f32, name="attp", tag="attp")
                        for jh in range(2):
                            for hc in range(2):
                                nc.tensor.matmul(
                                    attp[:, jh * 256 : (jh + 1) * 256],
                                    qTs[b][hc][:],
                                    kts[(jh, hc)][
                                        :,
                                        bb * 64 * JH
                                        + cx * 256 : bb * 64 * JH
                                        + (cx + 1) * 256,
                                    ],
                                    start=(hc == 0),
                                    stop=(hc == 1),
                                )
                        nc.vector.tensor_reduce(
                            out=accs[b][:, cx * 8 : (cx + 1) * 8],
                            in_=attp[:].rearrange(
                                "p (jh n j) -> p n jh j", jh=2, n=8
                            ),
                            axis=AX.XY,
                            op=MAX,
                        )

            # --- epilogue: max over i, then top-8 per batch
            col = opool.tile([N, BPC], f32)
            for b in range(BPC):
                accT = ps_sm.tile([N, L], f32, name="accT", tag="qt_ps")
                nc.tensor.transpose(accT[:], accs[b][:], ident[:L, :L])
                nc.vector.tensor_reduce(
                    out=col[:, b : b + 1], in_=accT[:], axis=AX.X, op=MAX
                )
            colT = ps_sm.tile([BPC, N], f32, name="colT", tag="qt_ps")
            nc.tensor.transpose(colT[:], col[:], ident[:N, :N])
            attb = opool.tile([BPC, N], f32)
            nc.scalar.copy(attb[:], colT[:])
            mx = opool.tile([BPC, TOPK], f32)
            ix = opool.tile([BPC, TOPK], mybir.dt.uint32)
            nc.vector.max(out=mx[:], in_=attb[:])
            nc.vector.max_index(out=ix[:], in_max=mx[:], in_values=attb[:])
            nc.gpsimd.dma_start(out=att_d[:, :], in_=attb[:])
            nc.gpsimd.dma_start(out=idx_d[:, :], in_=ix[:])

    nc.finalize()
    return nc


class _Runner:
    """Compile once, run many times: cached shard_map over the 8 cores."""

    def __init__(self, use_fp32r: bool, f32r_loads: bool = False):
        import jax
        import jax.core
        from jax.experimental.shard_map import shard_map
        from jax.sharding import Mesh, PartitionSpec

        import concourse.mybir as mybir
        from concourse import bass2jax

        self.nc = _build_nc(use_fp32r, f32r_loads)
        bass2jax.install_neuronx_cc_hook()

        partition_name = (
            self.nc.partition_id_tensor.name if self.nc.partition_id_tensor else None
        )
        in_names, out_names, out_avals = [], [], []
        for alloc in self.nc.m.functions[0].allocations:
            if not isinstance(alloc, mybir.MemoryLocationSet):
                continue
            name = alloc.memorylocations[0].name
            if alloc.kind == "ExternalInput":
                if name != partition_name:
                    in_names.append(name)
            elif alloc.kind == "ExternalOutput":
                out_names.append(name)
                out_avals.append(
                    jax.core.ShapedArray(
                        tuple(alloc.tensor_shape), mybir.dt.np(alloc.dtype)
                    )
                )
        self.in_names = in_names
        self.out_names = out_names
        self.out_shapes = [tuple(a.shape) for a in out_avals]
        self.out_dtypes = [a.dtype for a in out_avals]

        names_all = tuple(
            in_names + out_names + ([partition_name] if partition_name else [])
        )
        out_avals_t = tuple(out_avals)
        n_params = len(in_names)
        donate = tuple(range(n_params, n_params + len(out_names)))
        nc = self.nc

        def _body(*args):
            operands = list(args)
            if partition_name is not None:
                operands.append(bass2jax.partition_id_tensor())
            outs = bass2jax._bass_exec_p.bind(
                *operands,
                out_avals=out_avals_t,
                in_names=names_all,
                out_names=tuple(out_names),
                lowering_input_output_aliases=(),
                sim_require_finite=True,
                sim_require_nnan=True,
                nc=nc,
            )
            return tuple(outs)

        devices = jax.devices()[:NCORES]
        assert len(devices) == NCORES, f"need {NCORES} cores, saw {len(devices)}"
        mesh = Mesh(np.asarray(devices), ("core",))
        in_specs = (PartitionSpec("core"),) * (n_params + len(out_names))
        out_specs = (PartitionSpec("core"),) * len(out_names)
        self.fn = jax.jit(
            shard_map(
                _body,
                mesh=mesh,
                in_specs=in_specs,
                out_specs=out_specs,
                check_rep=False,
            ),
            donate_argnums=donate,
            keep_unused=True,
        )

    def __call__(self, in_maps):
        concat_in = [
            np.concatenate([np.asarray(m[name]) for m in in_maps], axis=0)
            for name in self.in_names
        ]
        concat_zeros = [
            np.zeros((NCORES * s[0], *s[1:]), d)
            for s, d in zip(self.out_shapes, self.out_dtypes)
        ]
        outs = self.fn(*concat_in, *concat_zeros)
        return [
            {
                name: np.asarray(outs[i]).reshape(NCORES, *self.out_shapes[i])[c]
                for i, name in enumerate(self.out_names)
            }
            for c in range(NCORES)
        ]


def _get_runner():
    key = (_use_fp32r(), _use_f32r_loads())
    if key not in _RUNNERS:
        _RUNNERS[key] = _Runner(*key)
    return _RUNNERS[key]


def make_in_maps(query, keys):
    """Shard full inputs batch-wise into the 8 per-core input dicts."""
    q = np.asarray(query, dtype=np.float32).reshape(L, BSZ, H)
    k = np.asarray(keys, dtype=np.float32).reshape(N, BSZ, L, H)
    in_maps = []
    for c in range(NCORES):
        sl = slice(c * BPC, (c + 1) * BPC)
        in_maps.append(
            {
                "q": np.ascontiguousarray(q[:, sl, :]),
                "k": np.ascontiguousarray(k[:, sl, :, :].transpose(1, 0, 2, 3)),
            }
        )
    return in_maps


def kernel(query, keys, values=None, **_unused):
    """Full-input entry point: returns (att (32,1,64) f32, topk_idx (8,32) i32)."""
    del values  # dead code in the reference module: never read
    runner = _get_runner()
    res = runner(make_in_maps(query, keys))
    att = np.concatenate([r["att"] for r in res], axis=0).reshape(BSZ, 1, N)
    idx = (
        np.concatenate([r["idx"] for r in res], axis=0)
        .astype(np.int32)
        .T.copy()
    )
    return att, idx
